# revision 91
# baseline (speedup 1.0000x reference)
"""Chamfer loss kernel for Trainium2 (8 NeuronCores, batch-sharded), v8.

Reference computation (per batch b):
    dist2[n, m] = sum_{c in 1..3} ((p_re[b,n,c]-q_re[b,m,c])^2
                                 + (p_im[b,n,c]-q_im[b,m,c])^2)
    loss = sum_b ( sum_n min_m dist2 + sum_m min_n dist2 )

Both norms fold into an 8-row matmul contraction so psum[n, m] = -dist2/2
(operand rows k=0..5 carry the 6 real components, rows 6/7 carry -0.5
consts and |.|^2 norms, staggered between the p and q sides).
loss = -2 * sum(all free-axis maxes of psum); the final scale + sum runs
on the host from a [128, 64] max table.

v8: the matmul operand tensors (Rext layout [32q+k, 256g+n]) are built on
the HOST (a pure reshape/transpose of the shard + tiny norm precompute)
and DMA'd in directly -- the whole on-device prologue of v4-v7 (PE
transposes, stage-1/2 copies, squares, norm reduces, zero-fills) is gone.
Matmuls start as soon as the first operand DMA lands (~4us vs ~8us).

Drains (the real cost): 16 batches x [128, 1024] psum.  Only ACT and DVE
can read PSUM (one PSUM operand per instruction), only DVE can max-fold
(the HW compiler rejects max on GpSimd), so per-batch patterns are:
  S = DVE direct grouped reduce from psum              (DVE ~1192ns)
  V = ACT copy psum->f16 SBUF; DVE fold1+fold2+reduce  (ACT ~1038, DVE ~848)
balanced so ACT and DVE finish together.  acc [128, 64] maxes DMA out in
4 chunks (early chunks hide the ~2.5us DMA latency); host sums * -2.

PE rides its p-state clock ramp via warm-up matmuls on a stub tile at t=0.
"""

import contextlib

import numpy as np

import concourse.bass as bass
import concourse.tile as tile
from concourse import bacc, mybir
from concourse.bass_utils import run_bass_kernel_spmd

N_CORES = 8
B_FULL = 128
BL = B_FULL // N_CORES  # 16 local batches per core
NPT = 256
F32 = mybir.dt.float32
F32R = mybir.dt.float32r
F16 = mybir.dt.float16

KNORM = {"p": 7, "q": 6}
KCONST = {"p": 6, "q": 7}

# per-batch drain pattern, 16 chars of S/V (see module docstring)
PATTERNS = "SVVVVSVVVVSVVVVS"
N_WARMUP = 2  # f32 warm-up matmuls at t=0 to ride the PE clock ramp


def _build_program():
    nc = bacc.Bacc("TRN2", target_bir_lowering=False, debug=False)
    r_d = nc.dram_tensor("rext", [128, 2048], F32R, kind="ExternalInput").ap()
    out_d = nc.dram_tensor("out", [128, 64], F32, kind="ExternalOutput").ap()

    with tile.TileContext(nc) as tc, contextlib.ExitStack() as ctx:
        consts = ctx.enter_context(tc.tile_pool(name="consts", bufs=1))
        ops = ctx.enter_context(tc.tile_pool(name="ops", bufs=1))
        dist_pool = ctx.enter_context(tc.tile_pool(name="dist", bufs=4, space="PSUM"))
        hpool = ctx.enter_context(tc.tile_pool(name="hp", bufs=3))

        R = ops.tile([128, 2048], F32R, name="Rext")
        Rext = {"p": R[:, 0:1024], "q": R[:, 1024:2048]}

        # operand DMAs: one packed tensor carries both sides, so a single
        # 2-run strided DMA delivers exactly the columns a group needs from
        # BOTH sides at once -- first matmuls start ~0.7us earlier than two
        # per-side DMAs.  g0 first (batches 0-3), then g1, then g2+g3.
        rv = R[:].rearrange("p (s c) -> p s c", s=2)
        dv = r_d.rearrange("p (s c) -> p s c", s=2)
        nc.sync.dma_start(out=rv[:, :, 0:256], in_=dv[:, :, 0:256])
        nc.sync.dma_start(out=rv[:, :, 256:512], in_=dv[:, :, 256:512])
        nc.sync.dma_start(out=rv[:, :, 512:1024], in_=dv[:, :, 512:1024])

        acc = consts.tile([128, 64], F32, name="acc")

        # PE warm-up: matmuls on a stub tile keep the clock ramp moving
        # while the operand DMAs land
        stub = consts.tile([8, 256], F32, name="stub")
        nc.gpsimd.memset(stub[:], 1.0)
        for w in range((N_WARMUP + 3) // 4):
            wm = dist_pool.tile([128, 1024], F32, tag="ps")
            for j in range(min(4, N_WARMUP - 4 * w)):
                nc.tensor.matmul(
                    wm[:, 256 * j: 256 * j + 256],
                    stub[0:8, 0:128], stub[0:8, 0:256],
                    start=True, stop=True, tile_position=(0, 0),
                )

        # ---- per-batch matmuls + drain ----
        def matmuls(b):
            g, qslot = b // 4, b % 4
            dist = dist_pool.tile([128, 1024], F32, tag="ps")
            for o in range(2):
                lo = 0 if o == 0 else 1024      # lhs side offset (p / q)
                ro = 1024 - lo
                for ch in range(2):
                    nc.tensor.matmul(
                        dist[:, 512 * o + 256 * ch: 512 * o + 256 * ch + 256],
                        R[32 * qslot: 32 * qslot + 8,
                          lo + 256 * g + 128 * ch: lo + 256 * g + 128 * ch + 128],
                        R[32 * qslot: 32 * qslot + 8,
                          ro + 256 * g: ro + 256 * g + 256],
                        start=True, stop=True,
                        tile_position=(32 * qslot, 0),
                    )
            return dist

        def s_batch(b):
            dist = matmuls(b)
            nc.vector.tensor_reduce(
                out=acc[:, 4 * b: 4 * b + 4],
                in_=dist[:].rearrange("p (s m) -> p s m", s=4),
                axis=mybir.AxisListType.X, op=mybir.AluOpType.max,
            )

        def v_pair(b1, split_fold1=False):
            # two adjacent V batches share one f16 tile; folds run once at
            # double width, amortizing per-op SBUF-access inits on DVE.
            # split_fold1 runs fold1 per batch half so the first half can
            # start as soon as the first ACT copy lands (used for the final
            # pair, where the second copy is on the critical path).
            h1 = hpool.tile([128, 2048], F16, tag="h1")
            h2 = hpool.tile([128, 1024], F16, tag="h2")
            for i, b in enumerate((b1, b1 + 1)):
                dist = matmuls(b)
                nc.scalar.copy(h1[:, 1024 * i: 1024 * i + 1024], dist[:])
                if split_fold1:
                    vh = h1[:, 1024 * i: 1024 * i + 1024].rearrange(
                        "p (s h m) -> p s h m", s=4, h=2)
                    nc.vector.tensor_tensor(
                        out=h2[:, 512 * i: 512 * i + 512].rearrange(
                            "p (s m) -> p s m", s=4),
                        in0=vh[:, :, 0], in1=vh[:, :, 1],
                        op=mybir.AluOpType.max,
                    )
            if not split_fold1:
                v = h1[:].rearrange("p (s h m) -> p s h m", s=8, h=2)
                nc.vector.tensor_tensor(
                    out=h2[:].rearrange("p (s m) -> p s m", s=8),
                    in0=v[:, :, 0], in1=v[:, :, 1], op=mybir.AluOpType.max,
                )
            h3 = hpool.tile([128, 512], F16, tag="h3")
            w = h2[:].rearrange("p (s h m) -> p s h m", s=8, h=2)
            nc.vector.tensor_tensor(
                out=h3[:].rearrange("p (s m) -> p s m", s=8),
                in0=w[:, :, 0], in1=w[:, :, 1], op=mybir.AluOpType.max,
            )
            h4 = hpool.tile([128, 256], F16, tag="h4")
            w4 = h3[:].rearrange("p (s h m) -> p s h m", s=8, h=2)
            nc.vector.tensor_tensor(
                out=h4[:].rearrange("p (s m) -> p s m", s=8),
                in0=w4[:, :, 0], in1=w4[:, :, 1], op=mybir.AluOpType.max,
            )
            nc.vector.tensor_reduce(
                out=acc[:, 4 * b1: 4 * b1 + 8],
                in_=h4[:].rearrange("p (s m) -> p s m", s=8),
                axis=mybir.AxisListType.X, op=mybir.AluOpType.max,
            )

        def v_single(b, split_copy=False):
            dist = matmuls(b)
            h1 = hpool.tile([128, 1024], F16, tag="h1s")
            h2 = hpool.tile([128, 512], F16, tag="h2s")
            if split_copy:
                # half-copies each followed by a half-fold: DVE work starts
                # ~0.6us earlier (fills its early-pipeline bubble) at the
                # cost of one extra ACT init
                for i in range(2):
                    nc.scalar.copy(h1[:, 512 * i: 512 * i + 512],
                                   dist[:, 512 * i: 512 * i + 512])
                    vh = h1[:, 512 * i: 512 * i + 512].rearrange(
                        "p (s h m) -> p s h m", s=2, h=2)
                    nc.vector.tensor_tensor(
                        out=h2[:, 256 * i: 256 * i + 256].rearrange(
                            "p (s m) -> p s m", s=2),
                        in0=vh[:, :, 0], in1=vh[:, :, 1],
                        op=mybir.AluOpType.max,
                    )
            else:
                nc.scalar.copy(h1[:], dist[:])
                v = h1[:].rearrange("p (s h m) -> p s h m", s=4, h=2)
                nc.vector.tensor_tensor(
                    out=h2[:].rearrange("p (s m) -> p s m", s=4),
                    in0=v[:, :, 0], in1=v[:, :, 1], op=mybir.AluOpType.max,
                )
            h3 = hpool.tile([128, 256], F16, tag="h3s")
            w = h2[:].rearrange("p (s h m) -> p s h m", s=4, h=2)
            nc.vector.tensor_tensor(
                out=h3[:].rearrange("p (s m) -> p s m", s=4),
                in0=w[:, :, 0], in1=w[:, :, 1], op=mybir.AluOpType.max,
            )
            nc.vector.tensor_reduce(
                out=acc[:, 4 * b: 4 * b + 4],
                in_=h3[:].rearrange("p (s m) -> p s m", s=4),
                axis=mybir.AxisListType.X, op=mybir.AluOpType.max,
            )

        def run_batches(lo, hi):
            b = lo
            while b < hi:
                if PATTERNS[b] == "S":
                    s_batch(b)
                    b += 1
                elif b == 1:
                    # de-paired: its fold chain starts right after its own
                    # ACT copy, filling DVE's early-pipeline bubble
                    v_single(b)
                    b += 1
                elif b + 1 < hi and PATTERNS[b + 1] == "V":
                    v_pair(b, split_fold1=(b == 13))
                    b += 2
                else:
                    v_single(b)
                    b += 1

        run_batches(0, 8)
        # early result chunks ship mid-kernel, hiding DMA latency
        nc.sync.dma_start(out=out_d[:, 0:32], in_=acc[:, 0:32])
        run_batches(8, 13)
        nc.sync.dma_start(out=out_d[:, 32:52], in_=acc[:, 32:52])
        run_batches(13, 15)
        nc.sync.dma_start(out=out_d[:, 52:60], in_=acc[:, 52:60])
        run_batches(15, 16)
        nc.sync.dma_start(out=out_d[:, 60:64], in_=acc[:, 60:64])

    nc.compile()
    return nc


_CACHE = {}


def _get_program():
    if "nc" not in _CACHE:
        _CACHE["nc"] = _build_program()
    return _CACHE["nc"]


def _build_rext(x, side):
    """Host-side operand layout: x [2, BL, 256, 4] -> R [128, 1024] f32.

    R[32*q + k, 256*g + n] for batch b = 4g+q:
      k=0..5: component (ci, r) -> k = 2*ci + r, value x[r, b, n, ci+1]
      k=KNORM[side]: |x_b,n|^2 (sum of the 6 squared components)
      k=KCONST[side]: -0.5
    Rows k=8..31 stay zero (matmuls never read them).
    """
    comp = x[:, :, :, 1:4]                 # [r, b, n, ci]
    comp = comp.transpose(1, 3, 0, 2)      # [b, ci, r, n]
    comp = np.ascontiguousarray(comp).reshape(BL, 6, NPT)
    norm = (comp.astype(np.float64) ** 2).sum(axis=1).astype(np.float32)
    R = np.zeros((128, 1024), dtype=np.float32)
    for b in range(BL):
        g, q = b // 4, b % 4
        R[32 * q: 32 * q + 6, 256 * g: 256 * g + 256] = comp[b]
        R[32 * q + KNORM[side], 256 * g: 256 * g + 256] = norm[b]
        R[32 * q + KCONST[side], 256 * g: 256 * g + 256] = -0.5
    return R


def make_in_maps(p, q):
    p = np.ascontiguousarray(np.asarray(p, dtype=np.float32))
    q = np.ascontiguousarray(np.asarray(q, dtype=np.float32))
    return [
        {
            "rext": np.ascontiguousarray(np.hstack([
                _build_rext(p[:, i * BL: (i + 1) * BL], "p"),
                _build_rext(q[:, i * BL: (i + 1) * BL], "q"),
            ])),
        }
        for i in range(N_CORES)
    ]


def kernel(p, q):
    nc = _get_program()
    in_maps = make_in_maps(p, q)
    res = run_bass_kernel_spmd(nc, in_maps, list(range(N_CORES)))
    total = 0.0
    for i in range(N_CORES):
        total += float(np.sum(res.results[i]["out"].astype(np.float64)))
    return np.float32(-2.0 * total)
